# revision 37
# baseline (speedup 1.0000x reference)
"""Trainium2 Bass kernel for nn_AsymmetricMultimodalSAE.

Sharding (8 NeuronCores):
 - Phase 1 (data-parallel over batch, 8 batches/core): gaussian view
   pooling runs with v_pad as the STATIONARY matmul operand so the fp32
   moving-cycle penalty lands on the 8 view columns and the pooled sums
   come out transposed ([D, rows]) - exactly the encoder's lhsT layout.
   Text pooling contracts t_pad against the mask vector as the moving
   operand (the mask multiply is free). Raw (unnormalized) pooled sums
   are AllGathered; the gaussian /den and the l2 norm fold away:
   x/||x|| == num/||num||, and 1/||num|| (computed post-gather via a
   ones-matmul partition contraction) is applied at the encoder PSUM
   drain as a per-partition scale.
 - Phase 2 (tensor-parallel over hidden 16384 -> 2048/core): exact-fp32
   encoder (float32r probed at 1.8e-4 rel error - would corrupt top-k
   selection, so full fp32 is used). Top-k is done WITHOUT indices:
   local top-32 values per 128-row chunk via 4x max8 + 3x match_replace,
   a per-chunk AllGather of the 8x32 candidate values, the global
   32nd-largest per row as threshold, then
   sparse = acts * (acts >= theta) in one scalar_tensor_tensor op.
   Dense latents are written directly as each core's column slice.
 - Decoder: bf16 partial matmul against host-pre-transposed bf16 weight
   shards (layout prep only; identical rounding to an on-device cast),
   bias added via an extra contraction row active on core 0 only,
   then ReduceScatter over rows (two v halves + text, overlapped).

Host side does sharding, layout prep, and concat/reshape assembly only.
Cost-model span ~489 us/core; correctness 2.3e-3 worst rel err (bf16
decoder rounding; latents and pooled outputs are ~1e-7).
"""
import os
import sys

sys.path.insert(0, "/opt/trn_rl_repo")

import numpy as np

import concourse.bass as bass
import concourse.mybir as mybir
import concourse.tile as tile
from concourse import bacc, masks
from concourse.bass_utils import run_bass_kernel_spmd

F32 = mybir.dt.float32
BF16 = mybir.dt.bfloat16
import ml_dtypes
_BF16NP = np.dtype(ml_dtypes.bfloat16)

NC = 8          # cores
B = 64          # batch
BL = B // NC    # batches per core
D = 1024        # feature dim
HH = 16384      # SAE hidden
HL = HH // NC   # hidden per core
T = 512         # text length
KV = 8          # views per batch
TOPK = 32
GAMMA = 10.0
EPS_POOL = 1e-6
EPS_NORM2 = 1e-24   # (1e-12)^2, clamp applied to squared norm
NEG = -1e30

RV = BL * KV          # view rows per core = 64
RT = BL               # text rows per core = 8
RL = RV + RT          # rows contributed per core = 72
NV = NC * RV          # total view rows = 512
NT = NC * RT          # total text rows = 64
NR = NV + NT          # total rows = 576
DC = D // 128         # 8
MCV = NV // 128       # 4 view row-chunks

_CACHE = {}


def _build(L, phase=99):
    """Build + compile the SPMD graph. L = H*W grid points (1024)."""
    assert L % 128 == 0 and L <= 1024
    LC = L // 128
    nc = bacc.Bacc("TRN2", target_bir_lowering=False, debug=False,
                   num_devices=NC)

    # ---- I/O ----
    vp = nc.dram_tensor("vp", [BL, L, D], F32, kind="ExternalInput")
    tp = nc.dram_tensor("tp", [BL, T, D], F32, kind="ExternalInput")
    tm = nc.dram_tensor("tm", [BL, T], F32, kind="ExternalInput")
    vf = nc.dram_tensor("vf", [4, RV], F32, kind="ExternalInput")
    gf = nc.dram_tensor("gf", [4, L], F32, kind="ExternalInput")
    encv = nc.dram_tensor("encv", [HL, D], F32, kind="ExternalInput")
    enct = nc.dram_tensor("enct", [HL, D], F32, kind="ExternalInput")
    decv = nc.dram_tensor("decv", [HL, D], BF16, kind="ExternalInput")
    dect = nc.dram_tensor("dect", [HL, D], BF16, kind="ExternalInput")
    dbv = nc.dram_tensor("dbv", [1, D], F32, kind="ExternalInput")
    dbt = nc.dram_tensor("dbt", [1, D], F32, kind="ExternalInput")
    bsel = nc.dram_tensor("bsel", [1, 1], F32, kind="ExternalInput")

    views_out = nc.dram_tensor("views_out", [RV, D], F32, kind="ExternalOutput")
    tg_out = nc.dram_tensor("tg_out", [RT, D], F32, kind="ExternalOutput")
    latv_out = nc.dram_tensor("latv_out", [NV, HL], F32, kind="ExternalOutput")
    latt_out = nc.dram_tensor("latt_out", [NT, HL], F32, kind="ExternalOutput")
    recv_out = nc.dram_tensor("recv_out", [NV // NC, D], F32,
                              kind="ExternalOutput")
    rect_out = nc.dram_tensor("rect_out", [NT // NC, D], F32,
                              kind="ExternalOutput")

    # collective buffers (internal DRAM)
    ccx_in = nc.dram_tensor("ccx_in", [D, RL], F32)
    xall = nc.dram_tensor("xall", [NC * D, RL], F32, addr_space="Shared")
    candv_in = [nc.dram_tensor(f"candv_in{mc}", [128, TOPK], F32)
                for mc in range(MCV)]
    candv_all = [nc.dram_tensor(f"candv_all{mc}", [NC * 128, TOPK], F32,
                                addr_space="Shared") for mc in range(MCV)]
    candt_in = nc.dram_tensor("candt_in", [NT, TOPK], F32)
    candt_all = nc.dram_tensor("candt_all", [NC * NT, TOPK], F32,
                               addr_space="Shared")
    rsv_in = [nc.dram_tensor(f"rsv_in{h}", [256, D], F32) for h in range(2)]
    rsv_out = [nc.dram_tensor(f"rsv_out{h}", [256 // NC, D], F32)
               for h in range(2)]
    rst_in = nc.dram_tensor("rst_in", [NT, D], F32)
    rst_out = nc.dram_tensor("rst_out", [NT // NC, D], F32)

    with tile.TileContext(nc) as tc:
        _body(tc, locals())
    nc.compile()
    return nc


def _get(L, phase=99):
    key = (L, phase)
    if key not in _CACHE:
        _CACHE[key] = _build(L, phase)
    return _CACHE[key]


def _body(tc, t):
    nc = tc.nc
    phase = t["phase"]
    AluOp = mybir.AluOpType
    Act = mybir.ActivationFunctionType
    rg = [list(range(NC))]
    L, LC = t["L"], t["LC"]
    vp, tp, tm, vf, gf = t["vp"], t["tp"], t["tm"], t["vf"], t["gf"]
    encv, enct, decv, dect = t["encv"], t["enct"], t["decv"], t["dect"]
    dbv, dbt, bsel = t["dbv"], t["dbt"], t["bsel"]
    views_out, tg_out = t["views_out"], t["tg_out"]
    latv_out, latt_out = t["latv_out"], t["latt_out"]
    recv_out, rect_out = t["recv_out"], t["rect_out"]
    ccx_in, xall = t["ccx_in"], t["xall"]
    candv_in, candv_all = t["candv_in"], t["candv_all"]
    candt_in, candt_all = t["candt_in"], t["candt_all"]
    rsv_in, rsv_out = t["rsv_in"], t["rsv_out"]
    rst_in, rst_out = t["rst_in"], t["rst_out"]

    pool = tc.alloc_tile_pool(name="const", bufs=1)
    stream = tc.alloc_tile_pool(name="stream", bufs=3)
    psum = tc.alloc_tile_pool(name="psum", bufs=2, space="PSUM")
    psumB = tc.alloc_tile_pool(name="psumB", bufs=4, space="PSUM")

    def release_all():
        psumB.release()
        psum.release()
        stream.release()
        pool.release()

    # ---- constants ----
    ident = pool.tile([128, 128], F32)
    masks.make_identity(nc, ident[:])
    identb = pool.tile([128, 128], BF16)
    masks.make_identity(nc, identb[:])
    ones_col = pool.tile([128, 1], F32)
    nc.vector.memset(ones_col[:], 1.0)
    ones_row_b = pool.tile([1, 128], BF16)
    nc.vector.memset(ones_row_b[:], 1.0)
    c_p2 = pool.tile([128, 1], F32)
    nc.vector.memset(c_p2[:], 2.0)
    c_m2 = pool.tile([128, 1], F32)
    nc.vector.memset(c_m2[:], -2.0)

    # ---- small inputs ----
    vf_sb = pool.tile([4, RV], F32)
    nc.sync.dma_start(out=vf_sb[:], in_=vf[:])
    gf_sb = pool.tile([4, L], F32)
    nc.sync.dma_start(out=gf_sb[:], in_=gf[:])
    tm_sb = pool.tile([RT, T], F32)
    nc.sync.dma_start(out=tm_sb[:], in_=tm[:])
    tmt = pool.tile([128, RT * (T // 128)], F32)
    nc.sync.dma_start(out=tmt[:], in_=tm.ap().rearrange(
        "b (k p) -> p (b k)", p=128))
    bsel_sb = pool.tile([1, 1], F32)
    nc.sync.dma_start(out=bsel_sb[:], in_=bsel[:])

    def bias_row(db_dram, name):
        db_f = pool.tile([1, D], F32, name=f"dbf_{name}")
        nc.sync.dma_start(out=db_f[:], in_=db_dram[:])
        nc.vector.tensor_scalar_mul(db_f[:], db_f[:], bsel_sb[:])
        db_b = pool.tile([1, D], BF16, name=f"dbb_{name}")
        nc.vector.tensor_copy(db_b[:], db_f[:])
        return db_b

    dbv_b = bias_row(dbv, "v")
    dbt_b = bias_row(dbt, "t")

    if phase <= 0:
        release_all()
        return

    # =================== phase 1: pooling (transposed outputs) ==========
    p1 = tc.alloc_tile_pool(name="p1", bufs=1)

    # m[j, l] and its row sums (for the v_views output scaling only)
    m_sb = p1.tile([RV, L], F32)
    for half in range(L // 512):
        pm = psum.tile([RV, 512], F32, tag="ps", name="pm")
        nc.tensor.matmul(pm[:], lhsT=vf_sb[:],
                         rhs=gf_sb[:, half * 512:(half + 1) * 512],
                         start=True, stop=True)
        nc.scalar.activation(m_sb[:, half * 512:(half + 1) * 512], pm[:],
                             Act.Exp)
    den = p1.tile([RV, 1], F32)
    nc.vector.tensor_reduce(out=den[:], in_=m_sb[:], axis=mybir.AxisListType.X,
                            op=AluOp.add)
    nc.vector.tensor_scalar_add(den[:], den[:], EPS_POOL)
    rden = p1.tile([RV, 1], F32)
    nc.vector.reciprocal(rden[:], den[:])
    tsum_r = p1.tile([RT, 1], F32)
    nc.vector.tensor_reduce(out=tsum_r[:], in_=tm_sb[:],
                            axis=mybir.AxisListType.X, op=AluOp.add)
    nc.vector.tensor_scalar_add(tsum_r[:], tsum_r[:], EPS_POOL)
    rt_r = p1.tile([RT, 1], F32)
    nc.vector.reciprocal(rt_r[:], tsum_r[:])
    # mT[p, lc*RV + j] = m[j, lc*128+p] (pooling rhs)
    mT = p1.tile([128, LC * RV], F32)
    for lc in range(LC):
        pmt = psum.tile([128, RV], F32, tag="ps", name="pmt")
        nc.tensor.matmul(pmt[:], lhsT=gf_sb[:, lc * 128:(lc + 1) * 128],
                         rhs=vf_sb[:], start=True, stop=True)
        nc.scalar.activation(mT[:, lc * RV:(lc + 1) * RV], pmt[:], Act.Exp)

    # xin_sb[p, dc*RL + j] = raw pooled sums, transposed.
    #   j < RV: view j (= bl*8+k); j >= RV: text batch j-RV.
    xin_sb = p1.tile([128, DC * RL], F32)
    xin3 = xin_sb[:].rearrange("p (dc j) -> p dc j", dc=DC)

    for bl in range(BL):
        pnb = psumB.tile([128, DC * KV], F32, tag="pb", name="pnb")
        for lc in range(LC):
            vt = p1.tile([128, D], F32, tag="vpt", bufs=4, name="vt")
            nc.sync.dma_start(out=vt[:], in_=vp[bl, lc * 128:(lc + 1) * 128, :])
            rhs = mT[:, lc * RV + bl * KV: lc * RV + (bl + 1) * KV]
            for dc in range(DC):
                nc.tensor.matmul(pnb[:, dc * KV:(dc + 1) * KV],
                                 lhsT=vt[:, dc * 128:(dc + 1) * 128],
                                 rhs=rhs, start=(lc == 0 and dc == 0),
                                 stop=(lc == LC - 1 and dc == DC - 1))
        nc.scalar.copy(
            xin3[:, :, bl * KV:(bl + 1) * KV],
            pnb[:].rearrange("p (dc k) -> p dc k", dc=DC))

    # text: tsumT accumulated with mask as moving operand
    for bl in range(BL):
        ptx = psumB.tile([128, DC], F32, tag="pb", name="ptx")
        for kc in range(T // 128):
            tt = p1.tile([128, D], F32, tag="tpt", bufs=4, name="tt")
            nc.sync.dma_start(out=tt[:], in_=tp[bl, kc * 128:(kc + 1) * 128, :])
            msk = tmt[:, bl * 4 + kc: bl * 4 + kc + 1]
            for dc in range(DC):
                nc.tensor.matmul(ptx[:, dc:dc + 1],
                                 lhsT=tt[:, dc * 128:(dc + 1) * 128],
                                 rhs=msk, start=(kc == 0 and dc == 0),
                                 stop=(kc == 3 and dc == DC - 1))
        nc.scalar.copy(xin3[:, :, RV + bl: RV + bl + 1],
                       ptx[:].rearrange("p (dc o) -> p dc o", dc=DC))

    # v_views / t_global outputs: transpose numT back, scale by 1/den
    views_sb = p1.tile([RV, D], F32)
    tg_sb = p1.tile([RT, D], F32)
    for dc in range(DC):
        pvo = psum.tile([RV, 128], F32, tag="ps", name="pvo")
        nc.tensor.transpose(pvo[:], xin3[:, dc, 0:RV], ident[:])
        nc.scalar.mul(views_sb[:, dc * 128:(dc + 1) * 128], pvo[:], rden[:])
        pto = psum.tile([RT, 128], F32, tag="ps", name="pto")
        nc.tensor.transpose(pto[:], xin3[:, dc, RV:RL], ident[:])
        nc.scalar.mul(tg_sb[:, dc * 128:(dc + 1) * 128], pto[:], rt_r[:])
    nc.sync.dma_start(out=views_out[:], in_=views_sb[:])
    nc.sync.dma_start(out=tg_out[:], in_=tg_sb[:])

    # bounce transposed raw sums out + AllGather
    nc.sync.dma_start(
        out=ccx_in.ap().rearrange("(dc p) j -> p dc j", p=128),
        in_=xin3[:, :, :])
    nc.gpsimd.collective_compute(
        "AllGather", AluOp.bypass, replica_groups=rg,
        ins=[ccx_in[:].opt()], outs=[xall[:].opt()])

    p1.release()
    if phase <= 1:
        release_all()
        return

    # ======== phase 2: gather Xt slices + row norms ========
    actp = tc.alloc_tile_pool(name="acts", bufs=1)
    acts_v = [actp.tile([128, HL], F32, name=f"actsv{i}") for i in range(MCV)]
    acts_t = actp.tile([NT, HL], F32)

    xtp = tc.alloc_tile_pool(name="xt", bufs=1)
    # xall: [(r dc p), j] ; Xt_v[dc][p, r*RV+j] , Xt_t[dc][p, r*RT+j]
    xall4 = xall.ap().rearrange("(r dc p) j -> dc p r j", r=NC, p=128)
    xtv = [xtp.tile([128, NV], F32, name=f"xtv{dc}") for dc in range(DC)]
    xtt = xtp.tile([128, DC * NT], F32)
    for dc in range(DC):
        nc.sync.dma_start(
            out=xtv[dc][:].rearrange("p (r j) -> p r j", r=NC),
            in_=xall4[dc, :, :, 0:RV])
        nc.sync.dma_start(
            out=xtt[:, dc * NT:(dc + 1) * NT].rearrange(
                "p (r j) -> p r j", r=NC),
            in_=xall4[dc, :, :, RV:RL])

    # row norms: ssq via squares + ones-matmul partition contraction
    pnv = psumB.tile([1, NV], F32, tag="pb", name="pnv")
    pnt = psumB.tile([1, NT], F32, tag="pb", name="pnt")
    for dc in range(DC):
        sqv = stream.tile([128, NV], F32, tag="sq", name="sqv")
        nc.vector.tensor_mul(sqv[:], xtv[dc][:], xtv[dc][:])
        nc.tensor.matmul(pnv[:], lhsT=ones_col[:], rhs=sqv[:],
                         start=(dc == 0), stop=(dc == DC - 1))
        sqt = stream.tile([128, NT], F32, tag="sqt", name="sqt")
        nc.vector.tensor_mul(sqt[:], xtt[:, dc * NT:(dc + 1) * NT],
                             xtt[:, dc * NT:(dc + 1) * NT])
        nc.tensor.matmul(pnt[:], lhsT=ones_col[:], rhs=sqt[:],
                         start=(dc == 0), stop=(dc == DC - 1))

    def rn_from_ssq(pssq, n, name):
        ssq = xtp.tile([1, n], F32, name=f"ssqr_{name}")
        nc.vector.tensor_scalar_max(ssq[:], pssq[:], EPS_NORM2)
        nrm = xtp.tile([1, n], F32, name=f"nrmr_{name}")
        nc.scalar.sqrt(nrm[:], ssq[:])
        rn = xtp.tile([1, n], F32, name=f"rnr_{name}")
        nc.vector.reciprocal(rn[:], nrm[:])
        cols = []
        for i in range(0, n, 128):
            w = min(128, n - i)
            pc = psum.tile([128, 1], F32, tag="ps", name="pc")
            nc.tensor.transpose(pc[0:w, :], rn[:, i:i + w], ident[0:1, 0:1])
            col = xtp.tile([128, 1], F32, name=f"rncol_{name}{i}")
            nc.vector.tensor_copy(col[0:w, :], pc[0:w, :])
            cols.append(col)
        return cols

    rnv_cols = rn_from_ssq(pnv, NV, "v")
    rnt_cols = rn_from_ssq(pnt, NT, "t")

    # ======== phase 3: encoders (+ acts transform) ========
    HG = 4
    HGS = HL // HG   # 512
    wpool = tc.alloc_tile_pool(name="wstream", bufs=2)

    def encoder(enc_in, acts_tiles, nrows_list, lhsT_fn, rn_cols):
        for hg in range(HG):
            wts = []
            for t4 in range(4):
                et = wpool.tile([128, D], F32, tag=f"et{t4}", bufs=4,
                                name=f"et{t4}")
                nc.sync.dma_start(
                    out=et[:],
                    in_=enc_in[hg * HGS + t4 * 128: hg * HGS + (t4 + 1) * 128, :])
                esc = wpool.tile([128, D], F32, tag="esc", bufs=1, name="esc")
                essq = wpool.tile([128, 1], F32, tag="essq", name="essq")
                nc.scalar.activation(esc[:], et[:], Act.Square,
                                     accum_out=essq[:])
                nc.vector.tensor_scalar_max(essq[:], essq[:], EPS_NORM2)
                enrm = wpool.tile([128, 1], F32, tag="enrm", name="enrm")
                nc.scalar.sqrt(enrm[:], essq[:])
                ern = wpool.tile([128, 1], F32, tag="ern", name="ern")
                nc.vector.reciprocal(ern[:], enrm[:])
                nc.vector.tensor_scalar_mul(et[:], et[:], ern[:])
                wts.append(et)
            wT = wpool.tile([128, DC * HGS], F32, tag="wT", name="wT")
            for dc in range(DC):
                pw = psum.tile([128, 512], F32, tag="ps", name="pw")
                for t4 in range(4):
                    nc.tensor.transpose(pw[:, t4 * 128:(t4 + 1) * 128],
                                        wts[t4][:, dc * 128:(dc + 1) * 128],
                                        ident[:])
                nc.scalar.copy(wT[:, dc * HGS:(dc + 1) * HGS], pw[:])
            for mc, nrows in enumerate(nrows_list):
                pe = psumB.tile([128, 512], F32, tag="pb", name="pe")
                for dc in range(DC):
                    nc.tensor.matmul(pe[0:nrows, :],
                                     lhsT=lhsT_fn(dc, mc, nrows),
                                     rhs=wT[:, dc * HGS:(dc + 1) * HGS],
                                     start=(dc == 0), stop=(dc == DC - 1))
                aslice = acts_tiles[mc][:, hg * HGS:(hg + 1) * HGS]
                # cos = (pe * rn_x) clipped above at 1 (guards the sqrt)
                nc.vector.tensor_scalar(aslice, pe[0:nrows, :],
                                        rn_cols[mc][0:nrows, :], 1.0,
                                        op0=AluOp.mult, op1=AluOp.min)
                sscr = wpool.tile([128, HGS], F32, tag="sscr", name="sscr")
                nc.scalar.activation(sscr[0:nrows, :], aslice, Act.Sqrt,
                                     bias=c_p2[0:nrows, :],
                                     scale=c_m2[0:nrows, :])
                nc.scalar.activation(aslice, sscr[0:nrows, :], Act.Copy,
                                     bias=2.0, scale=-1.0)

    encoder(encv, acts_v, [128] * MCV,
            lambda dc, mc, nr: xtv[dc][:, mc * 128:(mc + 1) * 128], rnv_cols)
    encoder(enct, [acts_t], [NT],
            lambda dc, mc, nr: xtt[:, dc * NT:(dc + 1) * NT], rnt_cols)

    wpool.release()
    xtp.release()

    # ======== phase 4+5: dec prep, then per-mc topk->theta->mask->spT ====
    KC = HL // 128   # 16
    decp = tc.alloc_tile_pool(name="dec", bufs=1)

    if phase >= 4:
        decT_t = [decp.tile([128, D], BF16, tag=f"decTt{kc}", bufs=1,
                            name=f"decTt{kc}") for kc in range(KC)]
        for kc in range(KC):
            nc.sync.dma_start(out=decT_t[kc][:],
                              in_=dect[kc * 128:(kc + 1) * 128, :])
        decT_v = [decp.tile([128, D], BF16, tag=f"decTv{kc}", bufs=1,
                            name=f"decTv{kc}") for kc in range(KC)]
        for kc in range(KC):
            nc.sync.dma_start(out=decT_v[kc][:],
                              in_=decv[kc * 128:(kc + 1) * 128, :])
        spT = decp.tile([128, KC * NR], BF16, name="spT")
        spT3 = spT[:].rearrange("p (kc j) -> p kc j", kc=KC)

    # ---- topk / theta / mask / latent / sparse-transpose, per row-chunk
    def local_topk(A, nrows, cand_dram):
        cand = stream.tile([128, TOPK], F32, tag="cand", name="cand")
        scr = stream.tile([128, HL], F32, tag="topkscr", bufs=1, name="scr")
        nc.vector.max(out=cand[0:nrows, 0:8], in_=A)
        nc.vector.match_replace(out=scr[0:nrows, :],
                                in_to_replace=cand[0:nrows, 0:8],
                                in_values=A, imm_value=NEG)
        for r in range(1, 4):
            nc.vector.max(out=cand[0:nrows, 8 * r:8 * (r + 1)],
                          in_=scr[0:nrows, :])
            if r < 3:
                nc.vector.match_replace(
                    out=scr[0:nrows, :],
                    in_to_replace=cand[0:nrows, 8 * r:8 * (r + 1)],
                    in_values=scr[0:nrows, :], imm_value=NEG)
        nc.sync.dma_start(out=cand_dram[0:nrows, :], in_=cand[0:nrows, :])

    def theta_mask(A, nrows, call_dram, row0_lat, lat_dram):
        gc = stream.tile([128, NC * TOPK], F32, tag="gc", name="gc")
        src3 = call_dram.ap().rearrange("(r q) j -> q r j", r=NC)
        nc.sync.dma_start(
            out=gc[0:nrows, :].rearrange("q (r j) -> q r j", r=NC),
            in_=src3[:, :, :] if nrows == 128 else src3[0:nrows, :, :])
        c2 = stream.tile([128, TOPK], F32, tag="c2", name="c2")
        nc.vector.max(out=c2[0:nrows, 0:8], in_=gc[0:nrows, :])
        for r in range(1, 4):
            nc.vector.match_replace(out=gc[0:nrows, :],
                                    in_to_replace=c2[0:nrows, 8 * (r - 1):8 * r],
                                    in_values=gc[0:nrows, :], imm_value=NEG)
            nc.vector.max(out=c2[0:nrows, 8 * r:8 * (r + 1)], in_=gc[0:nrows, :])
        theta = c2[0:nrows, TOPK - 1:TOPK]
        nc.vector.scalar_tensor_tensor(out=A, in0=A, scalar=theta, in1=A,
                                       op0=AluOp.is_ge, op1=AluOp.mult)
        nc.sync.dma_start(out=lat_dram[row0_lat:row0_lat + nrows, :], in_=A)

    for mc in range(MCV):
        local_topk(acts_v[mc][:], 128, candv_in[mc])
        nc.gpsimd.collective_compute(
            "AllGather", AluOp.bypass, replica_groups=rg,
            ins=[candv_in[mc][:].opt()], outs=[candv_all[mc][:].opt()])
        theta_mask(acts_v[mc][:], 128, candv_all[mc], mc * 128, latv_out)
        if phase >= 4:
            # sparse transpose for this row block, all kc
            for g in range(4):
                ps = psum.tile([128, 512], F32, tag="ps", name="psT")
                for k in range(4):
                    kc = g * 4 + k
                    nc.tensor.transpose(ps[:, k * 128:(k + 1) * 128],
                                        acts_v[mc][:, kc * 128:(kc + 1) * 128],
                                        ident[:])
                nc.scalar.copy(
                    spT3[:, g * 4:(g + 1) * 4, mc * 128:(mc + 1) * 128],
                    ps[:].rearrange("p (k j) -> p k j", k=4))

    local_topk(acts_t[:], NT, candt_in)
    nc.gpsimd.collective_compute(
        "AllGather", AluOp.bypass, replica_groups=rg,
        ins=[candt_in[:].opt()], outs=[candt_all[:].opt()])
    theta_mask(acts_t[:], NT, candt_all, 0, latt_out)
    if phase <= 3:
        decp.release()
        actp.release()
        release_all()
        return

    for g in range(4):
        ps2 = psum.tile([128, 4 * NT], F32, tag="ps", name="psT2")
        for k in range(4):
            kc = g * 4 + k
            nc.tensor.transpose(ps2[:, k * NT:(k + 1) * NT],
                                acts_t[:, kc * 128:(kc + 1) * 128],
                                ident[0:NT, 0:NT])
        nc.scalar.copy(spT3[:, g * 4:(g + 1) * 4, NV:NR],
                       ps2[:].rearrange("p (k j) -> p k j", k=4))

    # ---- decoders ----
    MCS = [(mc, 128, decT_v, dbv_b, mc * 128) for mc in range(MCV)]
    MCS += [(MCV, NT, decT_t, dbt_b, 0)]
    for mi, nrows, decT, db_b, row0 in MCS:
        col0 = row0 if mi < MCV else NV
        recon_sb = decp.tile([128, D], F32, tag="recon", bufs=3, name="recon_sb")
        for half in range(2):
            pr = psumB.tile([128, 512], F32, tag="pb", name="pr")
            for kc in range(KC):
                nc.tensor.matmul(
                    pr[0:nrows, :],
                    lhsT=spT3[:, kc, col0:col0 + nrows],
                    rhs=decT[kc][:, half * 512:(half + 1) * 512],
                    start=(kc == 0), stop=False)
            nc.tensor.matmul(pr[0:nrows, :], lhsT=ones_row_b[:, 0:nrows],
                             rhs=db_b[:, half * 512:(half + 1) * 512],
                             start=False, stop=True)
            nc.scalar.copy(recon_sb[0:nrows, half * 512:(half + 1) * 512],
                           pr[0:nrows, :])
        if mi == MCV:
            nc.sync.dma_start(out=rst_in[:], in_=recon_sb[0:nrows, :])
            nc.gpsimd.collective_compute(
                "ReduceScatter", AluOp.add, replica_groups=rg,
                ins=[rst_in[:].opt()], outs=[rst_out[:].opt()])
            nc.sync.dma_start(out=rect_out[:], in_=rst_out[:])
        else:
            h = mi // 2
            nc.sync.dma_start(
                out=rsv_in[h][(mi % 2) * 128:(mi % 2 + 1) * 128, :],
                in_=recon_sb[0:nrows, :])
            if mi % 2 == 1:
                nc.gpsimd.collective_compute(
                    "ReduceScatter", AluOp.add, replica_groups=rg,
                    ins=[rsv_in[h][:].opt()], outs=[rsv_out[h][:].opt()])
                nc.sync.dma_start(
                    out=recv_out[h * 32:(h + 1) * 32, :],
                    in_=rsv_out[h][:])
    decp.release()
    actp.release()
    release_all()


def _prep_inputs(v_pad, t_pad, t_mask, centers, enc_v, dec_v_w, dec_v_b,
                 enc_t, dec_t_w, dec_t_b, H, W):
    L = H * W
    y = (np.arange(H, dtype=np.float32) + 0.5) / H
    x = (np.arange(W, dtype=np.float32) + 0.5) / W
    gy, gx = np.meshgrid(y, x, indexing="ij")
    gxf = gx.ravel().astype(np.float32)
    gyf = gy.ravel().astype(np.float32)
    gfm = np.stack([gxf, gyf, np.ones_like(gxf),
                    -GAMMA * (gxf * gxf + gyf * gyf)], axis=0)  # [4, L]
    in_maps = []
    for c in range(NC):
        cb = centers[c * BL:(c + 1) * BL]          # [BL, K, 2]
        cx = cb[..., 0].reshape(-1).astype(np.float32)
        cy = cb[..., 1].reshape(-1).astype(np.float32)
        vfm = np.stack([2 * GAMMA * cx, 2 * GAMMA * cy,
                        -GAMMA * (cx * cx + cy * cy),
                        np.ones_like(cx)], axis=0)  # [4, RV]
        in_maps.append({
            "vp": np.ascontiguousarray(v_pad[c * BL:(c + 1) * BL, :L, :]),
            "tp": np.ascontiguousarray(t_pad[c * BL:(c + 1) * BL]),
            "tm": np.ascontiguousarray(t_mask[c * BL:(c + 1) * BL]),
            "vf": np.ascontiguousarray(vfm),
            "gf": np.ascontiguousarray(gfm),
            "encv": np.ascontiguousarray(enc_v[c * HL:(c + 1) * HL]),
            "enct": np.ascontiguousarray(enc_t[c * HL:(c + 1) * HL]),
            "decv": np.ascontiguousarray(
                dec_v_w[:, c * HL:(c + 1) * HL].T).astype(_BF16NP),
            "dect": np.ascontiguousarray(
                dec_t_w[:, c * HL:(c + 1) * HL].T).astype(_BF16NP),
            "dbv": np.ascontiguousarray(dec_v_b.reshape(1, D)),
            "dbt": np.ascontiguousarray(dec_t_b.reshape(1, D)),
            "bsel": np.array([[1.0 if c == 0 else 0.0]], np.float32),
        })
    return in_maps


def kernel(v_pad, v_len, grid_thws, t_pad, t_mask, centers,
           enc_v, dec_v_w, dec_v_b, enc_t, dec_t_w, dec_t_b):
    v_pad = np.asarray(v_pad, np.float32)
    t_pad = np.asarray(t_pad, np.float32)
    t_mask = np.asarray(t_mask, np.float32)
    centers = np.asarray(centers, np.float32)
    enc_v = np.asarray(enc_v, np.float32)
    dec_v_w = np.asarray(dec_v_w, np.float32)
    dec_v_b = np.asarray(dec_v_b, np.float32)
    enc_t = np.asarray(enc_t, np.float32)
    dec_t_w = np.asarray(dec_t_w, np.float32)
    dec_t_b = np.asarray(dec_t_b, np.float32)
    H = int(np.asarray(grid_thws)[0, 1])
    W = int(np.asarray(grid_thws)[0, 2])
    L = H * W

    nc = _get(L, int(os.environ.get("BASSK_PHASE", "99")))
    in_maps = _prep_inputs(v_pad, t_pad, t_mask, centers, enc_v, dec_v_w,
                           dec_v_b, enc_t, dec_t_w, dec_t_b, H, W)
    trace = bool(int(os.environ.get("BASSK_TRACE", "0")))
    res = run_bass_kernel_spmd(nc, in_maps, core_ids=list(range(NC)),
                               trace=trace)
    if trace and getattr(res, "exec_time_ns", None) is not None:
        print(f"HW exec time: {res.exec_time_ns} ns")
        kernel.last_exec_time_ns = res.exec_time_ns
    r = res.results
    v_views = np.concatenate([r[c]["views_out"] for c in range(NC)], axis=0)
    t_global = np.concatenate([r[c]["tg_out"] for c in range(NC)], axis=0)
    latent_v = np.concatenate([r[c]["latv_out"] for c in range(NC)], axis=1)
    latent_t = np.concatenate([r[c]["latt_out"] for c in range(NC)], axis=1)
    # recv_out[c][h*32+i] = reduced rows h*256 + c*32 + i
    rv = np.stack([r[c]["recv_out"] for c in range(NC)])   # [NC, 64, D]
    rv = rv.reshape(NC, 2, 32, D)                          # [c, h, i, D]
    recon_v = rv.transpose(1, 0, 2, 3).reshape(NV, D)
    recon_t = np.concatenate([r[c]["rect_out"] for c in range(NC)], axis=0)
    return (recon_v.reshape(B, KV, D),
            v_views.reshape(B, KV, D),
            recon_t.reshape(B, D),
            t_global.reshape(B, D),
            latent_v.reshape(B, KV, HH),
            latent_t.reshape(B, HH))
